# revision 15
# baseline (speedup 1.0000x reference)
"""KNN anomaly-score kernel for Trainium2 (8 NeuronCores, Bass/Tile).

Problem: features [B=1024, D=768], memory_bank [N=50000, D=768], k=9.
anomaly_score[b] = mean of the k smallest Euclidean distances from
features[b] to the memory bank rows.

Strategy (per the sharding hint): shard memory-bank rows across the 8
cores.  Each core computes its [B, N/8] block of a per-row ranking score
v = f.m' on the TensorEngine in fp8e4m3 with perf_mode=DoubleRow (2
weights per PE cell, contraction 256 per matmul -> ~2x bf16 rate).

The norm terms are folded in without extra matmuls:
  - The per-row ||f||^2/2 term is constant within a row, so it does not
    affect per-row ranking; it is applied on the host.
  - The per-column ||m||^2/2 term is carried by sacrificing the last two
    of the 768 contraction dims: rows 0..765 hold the data, rows 766/767
    hold a hi/lo fp8 split of r = (768 - ||m||^2)/2 against features
    rows fixed at 1.0.  Dropping 2 of 768 data dims adds unbiased noise
    (sigma ~1.8 on v); together with fp8 quantization the end-to-end
    score error is ~2e-3 max (measured), well under the 2e-2 gate.
  So v = f[:766].m[:766] + r, and d^2 = x_sq + 768 - 2v on the host.

Selection: for each 1024-column block the DVE MAX8 instruction extracts
the block's top-8 v values (one pass).  The device returns all block
candidates [B, 8*nblocks]; the host gathers the 8 cores' candidates and
reduces to the global top-k.  A true top-k member can be missing only if
>=8 elements of its block rank above it, which forces >=8 of the
observed top-k to come from that single block - the host detects exactly
that condition and recomputes the affected rows with numpy, so the
selection is structurally sound for any k.
"""

import functools
import sys

sys.path.insert(0, "/opt/trn_rl_repo")

import numpy as np

P = 128
NCORES = 8
C2 = None  # set per-D at prep time: the constant pulled out of ||m||^2
PAD_V = -240.0  # fp8 value placed in the r rows of padding columns


def _ceil_to(x, m):
    return (x + m - 1) // m * m


def _chunk_widths(NPAD):
    """Column blocks per PSUM tile (each <=1024, >=8).  First block is
    small so the first matmuls start on minimal DMA; the last blocks are
    small so the post-matmul drain (copy+max8+out) is short."""
    assert NPAD >= 1024
    widths = [512]
    rem = NPAD - 512
    while rem > 1536:
        widths.append(1024)
        rem -= 1024
    if rem <= 519:
        widths.append(rem)
    elif rem <= 1024:
        widths += [512, rem - 512]
    elif rem < 1024 + 8:
        widths += [512, rem - 512]  # rem-512 in (512, 520): still <= 1024
    else:
        widths += [1024, rem - 1024]
    assert sum(widths) == NPAD and all(8 <= w <= 1024 for w in widths)
    return widths


@functools.lru_cache(maxsize=4)
def _build(B, D, NPAD):
    """Build (and finalize) the SPMD Bass module for one core's shard."""
    from contextlib import ExitStack

    import concourse.tile as tile
    from concourse import bacc, mybir

    f32 = mybir.dt.float32
    bf16 = mybir.dt.bfloat16
    f8 = mybir.dt.float8e4
    DR = mybir.MatmulPerfMode.DoubleRow

    KT = D // P
    KJ = KT // 2
    MT = B // P
    assert D % (2 * P) == 0 and B % P == 0 and NPAD >= 1024
    widths = _chunk_widths(NPAD)
    NCH = len(widths)
    CW = 8 * NCH  # candidates per row per core

    nc = bacc.Bacc(
        "TRN2", target_bir_lowering=False, debug=False, num_devices=NCORES
    )

    # host lays out both operands chunk-blocked and partition-contiguous
    # so DMAs move multi-KB whole-partition runs.  f is m-major so the
    # first matmul only needs the m=0 slice (~KT*P*P bytes).
    f_t = nc.declare_dram_parameter("f_t", [P, MT * KT * P], f8, isOutput=False)
    b_t = nc.declare_dram_parameter("b_t", [NCH, P, KT * 1024], f8, isOutput=False)
    out = nc.declare_dram_parameter("cand", [P, NCH * MT * 8], f32, isOutput=True)

    with tile.TileContext(nc) as tc, ExitStack() as ctx:
        cpool = ctx.enter_context(tc.tile_pool(name="const", bufs=1))
        # ring of 3 bank tiles: chunk ci's DMA is gated on chunk ci-3's
        # matmuls, so early chunks get the full DMA bandwidth instead of
        # fair-sharing with every later chunk's descriptors
        bpool = ctx.enter_context(tc.tile_pool(name="bank", bufs=3))
        ppool = ctx.enter_context(tc.tile_pool(name="psum", bufs=4, space="PSUM"))
        upool = ctx.enter_context(tc.tile_pool(name="u", bufs=6))

        b_t_view = b_t.rearrange("c p (kt n) -> c p kt n", n=1024)
        f_t_view = f_t.rearrange("p (mt kt b) -> p mt kt b", kt=KT, b=P)

        # PE warm-up during the initial DMA wait: garbage matmuls on a
        # zeroed tile get the HAM clock-gate toward 2.4GHz before real work
        warm = cpool.tile([P, 512], bf16, tag="warm")
        nc.vector.memset(warm[:], 0.0)
        wpsum = ppool.tile([P, 1024], f32, tag="pt")  # borrow a pt slot
        for _ in range(9):
            nc.tensor.matmul(
                wpsum[:, :512], lhsT=warm[:, :P], rhs=warm[:], start=True, stop=True
            )

        # features, m-major: [P, MT, KT, P]; m=0 lands first
        ftile = cpool.tile([P, MT, KT, P], f8, tag="ft")
        nc.scalar.dma_start(ftile[:, 0, :, :], f_t_view[:, 0, :, :])
        if MT > 1:
            nc.scalar.dma_start(ftile[:, 1:, :, :], f_t_view[:, 1:, :, :])

        # bank chunk DMAs; chunk 0 lands in kt-pair slices so the first
        # matmuls start as soon as its kj=0 slice is in
        btiles = []
        for ci in range(NCH):
            W = widths[ci]
            btile = bpool.tile([P, KT, 1024], f8, tag="bt", name=f"bt{ci}")
            if ci == 0:
                for kj in range(KJ):
                    nc.sync.dma_start(
                        btile[:, 2 * kj : 2 * kj + 2, :W],
                        b_t_view[ci, :, 2 * kj : 2 * kj + 2, :W],
                    )
            else:
                nc.sync.dma_start(
                    btile[:, :, :W], b_t_view[ci, :, :, :W]
                )
            btiles.append(btile)

        # candidates chunk-major: each chunk's slab DMAs out as soon as
        # its last MAX8 lands, overlapping output with compute
        cand = cpool.tile([P, NCH * MT * 8], f32, tag="cand")

        def chalves(W):
            out_, lo = [], 0
            while lo < W:
                out_.append((lo, min(512, W - lo)))
                lo += 512
            return out_

        for ci in range(NCH):
            W = widths[ci]
            for m in range(MT):
                pt = ppool.tile([P, 1024], f32, tag="pt", name=f"pt{ci}_{m}")
                for kj in range(KJ):
                    for hlo, hw in chalves(W):
                        nc.tensor.matmul(
                            pt[:, hlo : hlo + hw],
                            lhsT=ftile[:, m, 2 * kj : 2 * kj + 2, :],
                            rhs=btiles[ci][:, 2 * kj : 2 * kj + 2, hlo : hlo + hw],
                            start=(kj == 0),
                            stop=(kj == KJ - 1),
                            perf_mode=DR,
                        )
                u = upool.tile([P, 1024], f32, tag="u")
                nc.scalar.copy(u[:, :W], pt[:, :W])
                nc.vector.max(
                    cand[:, (ci * MT + m) * 8 : (ci * MT + m) * 8 + 8], u[:, :W]
                )
            # issue on the (idle) GpSimd queue: the Sync queue blocks on
            # ring-gated bank-chunk DMA issues until late in the run
            nc.gpsimd.dma_start(
                out[:, ci * MT * 8 : (ci + 1) * MT * 8],
                cand[:, ci * MT * 8 : (ci + 1) * MT * 8],
            )

    nc.finalize()
    return nc


def _host_prep(features, memory_bank):
    """Shard + lay out fp8 inputs for the 8 cores (chunk-blocked,
    partition-contiguous layouts so DMAs move multi-KB runs)."""
    import ml_dtypes

    f8 = ml_dtypes.float8_e4m3
    B, D = features.shape
    N = memory_bank.shape[0]
    DQ = D - 2  # data dims; last two carry the m-norm term
    KT = D // P
    MT = B // P
    NSH = -(-N // NCORES)
    NPAD = max(NSH, 1024)
    if NPAD % 1024 and NPAD % 1024 < 8:
        NPAD = _ceil_to(NPAD, 1024)
    widths = _chunk_widths(NPAD)
    NCH = len(widths)

    c2 = float(D)
    fT = np.empty((D, B), f8)
    fT[:DQ] = features[:, :DQ].T.astype(f8)
    fT[DQ:] = 1.0
    # [D, B] -> m-major [P, MT, KT, P]: element (p, m, kt, j) = fT[kt*P+p, m*P+j]
    f_t = np.ascontiguousarray(
        fT.reshape(KT, P, MT, P).transpose(1, 2, 0, 3).reshape(P, MT * KT * P)
    )
    x_sq = np.einsum("bd,bd->b", features, features, dtype=np.float32)

    msq = np.einsum("nd,nd->n", memory_bank, memory_bank, dtype=np.float32)
    r = 0.5 * (c2 - msq)
    r_hi = r.astype(f8)
    r_lo = (r - r_hi.astype(np.float32)).astype(f8)

    in_maps = []
    for i in range(NCORES):
        lo = i * NSH
        hi = min(lo + NSH, N)
        n_i = hi - lo
        bT = np.zeros((D, NPAD), f8)
        bT[:DQ, :n_i] = memory_bank[lo:hi, :DQ].T.astype(f8)
        bT[DQ, :n_i] = r_hi[lo:hi]
        bT[DQ + 1, :n_i] = r_lo[lo:hi]
        if n_i < NPAD:  # padding columns must rank below everything
            bT[DQ, n_i:] = PAD_V
            bT[DQ + 1, n_i:] = PAD_V
        # chunk-blocked: [NCH, P, KT*1024], unused tail cols left zero
        b_t = np.zeros((NCH, P, KT * 1024), f8)
        c0 = 0
        for ci, W in enumerate(widths):
            blk = bT[:, c0 : c0 + W].reshape(KT, P, W).transpose(1, 0, 2)
            b_t[ci].reshape(P, KT, 1024)[:, :, :W] = blk
            c0 += W
        in_maps.append({"f_t": f_t, "b_t": b_t})
    return in_maps, NPAD, x_sq, msq, c2


# test.py can flip these to get a profiled run
TRACE = False
LAST_RESULT = None
N_RECOMPUTED = 0


def _install_ntff_hook():
    """This container's `antenv` lacks `axon_hooks`; synthesize it so
    run_bass_kernel_spmd(trace=True) can profile via the axon .so."""
    import sys as _sys

    if "antenv.axon_hooks" in _sys.modules:
        return
    import contextlib, ctypes, types

    mod = types.ModuleType("antenv.axon_hooks")
    mod._hook = None
    mod.set_axon_ntff_profile_hook = lambda h: setattr(mod, "_hook", h)
    mod.get_axon_ntff_profile_hook = lambda: mod._hook

    so_path = "/opt/axon/libaxon_pjrt.so"
    try:
        lib = ctypes.CDLL(so_path)
        lib.axon_start_nrt_profile.argtypes = [
            ctypes.POINTER(ctypes.c_int64),
            ctypes.c_size_t,
        ]
        lib.axon_start_nrt_profile.restype = ctypes.c_int64
        lib.axon_stop_nrt_profile.argtypes = [ctypes.c_char_p]
        lib.axon_stop_nrt_profile.restype = ctypes.c_int64

        @contextlib.contextmanager
        def _hook(output_dir, device_ids):
            import jax

            jax.devices()
            if device_ids:
                ids = (ctypes.c_int64 * len(device_ids))(*device_ids)
                rc = lib.axon_start_nrt_profile(ids, len(device_ids))
            else:
                rc = lib.axon_start_nrt_profile(None, 0)
            if rc != 0:
                raise RuntimeError(f"axon_start_nrt_profile rc={rc}")
            try:
                yield
            finally:
                n = lib.axon_stop_nrt_profile(str(output_dir).encode())
                print(f"profile: {n} file(s) written to {output_dir}")

        mod._hook = _hook
    except (OSError, AttributeError):
        pass

    import antenv

    _sys.modules["antenv.axon_hooks"] = mod
    antenv.axon_hooks = mod


def _exact_row_scores(features, memory_bank, rows, kk):
    """Exact numpy top-k mean distance for a few suspect rows."""
    f = features[rows]  # [R, D]
    d2 = (
        np.einsum("rd,rd->r", f, f)[:, None]
        + np.einsum("nd,nd->n", memory_bank, memory_bank)[None, :]
        - 2.0 * (f @ memory_bank.T)
    )
    d2k = np.sort(d2, axis=1)[:, :kk]
    return np.sqrt(np.maximum(d2k, 0.0)).mean(axis=1)


def kernel(features, memory_bank, k):
    global LAST_RESULT, N_RECOMPUTED
    from concourse.bass_utils import run_bass_kernel_spmd

    features = np.asarray(features, dtype=np.float32)
    memory_bank = np.asarray(memory_bank, dtype=np.float32)
    B, D = features.shape
    N = memory_bank.shape[0]
    kk = min(int(k), N)
    if kk <= 0:
        # mean over an empty candidate set (matches jnp.mean of empty)
        return np.full(B, np.nan, np.float32)

    in_maps, NPAD, x_sq, msq, c2 = _host_prep(features, memory_bank)
    nc = _build(B, D, NPAD)

    if TRACE:
        _install_ntff_hook()
    res = run_bass_kernel_spmd(nc, in_maps, list(range(NCORES)), trace=TRACE)
    LAST_RESULT = res

    # gather per-(core, block) top-8 candidates; larger v = closer.
    # device layout is [P, NCH*MT*8]: row p, col (ci*MT+m)*8+j -> feature
    # row m*P+p, block ci, candidate j
    MT = B // P
    cols = []
    for i in range(NCORES):
        arr = np.asarray(res.results[i]["cand"], dtype=np.float32)
        NCH = arr.shape[1] // (MT * 8)
        cols.append(
            arr.reshape(P, NCH, MT, 8).transpose(2, 0, 1, 3).reshape(B, NCH * 8)
        )
    v = np.concatenate(cols, axis=1)  # [B, NCORES * 8 * nblocks]
    return _finalize(v, features, memory_bank, kk, x_sq, c2)


def _finalize(v, features, memory_bank, kk, x_sq, c2):
    """Reduce the per-(core, block) top-8 candidates to the final scores."""
    global N_RECOMPUTED
    kk_c = min(kk, v.shape[1])
    order = np.argsort(-v, axis=1)[:, :kk_c]  # observed top-k candidates
    vk = np.take_along_axis(v, order, axis=1)
    d = np.sqrt(np.maximum(x_sq[:, None] + c2 - 2.0 * vk, 0.0))
    scores = d.mean(axis=1).astype(np.float32)

    # A true top-k member can only be missing if >=8 elements of its
    # 1024-column block outrank it; then >=8 of the observed top-k come
    # from that block (index group of 8).  Recompute such rows exactly.
    N_RECOMPUTED = 0
    if kk >= 9:
        if kk > v.shape[1]:  # more than the candidate pool: all rows exact
            suspects = np.arange(v.shape[0])
        else:
            grp = np.sort(order // 8, axis=1)
            same8 = (grp[:, 7:] == grp[:, : grp.shape[1] - 7]).any(axis=1)
            suspects = np.nonzero(same8)[0]
        if suspects.size:
            N_RECOMPUTED = suspects.size
            scores[suspects] = _exact_row_scores(
                features, memory_bank, suspects, kk
            ).astype(np.float32)

    return scores


# revision 17
# speedup vs baseline: 1.0198x; 1.0198x over previous
"""KNN anomaly-score kernel for Trainium2 (8 NeuronCores, Bass/Tile).

Problem: features [B=1024, D=768], memory_bank [N=50000, D=768], k=9.
anomaly_score[b] = mean of the k smallest Euclidean distances from
features[b] to the memory bank rows.

Strategy (per the sharding hint): shard memory-bank rows across the 8
cores.  Each core computes its [B, N/8] block of a per-row ranking score
v = f.m' on the TensorEngine in fp8e4m3 with perf_mode=DoubleRow (2
weights per PE cell, contraction 256 per matmul -> ~2x bf16 rate).

The norm terms are folded in without extra matmuls:
  - The per-row ||f||^2/2 term is constant within a row, so it does not
    affect per-row ranking; it is applied on the host.
  - The per-column ||m||^2/2 term is carried by sacrificing the last two
    of the 768 contraction dims: rows 0..765 hold the data, rows 766/767
    hold a hi/lo fp8 split of r = (768 - ||m||^2)/2 against features
    rows fixed at 1.0.  Dropping 2 of 768 data dims adds unbiased noise
    (sigma ~1.8 on v); together with fp8 quantization the end-to-end
    score error is ~2e-3 max (measured), well under the 2e-2 gate.
  So v = f[:766].m[:766] + r, and d^2 = x_sq + 768 - 2v on the host.

Selection: for each 1024-column block the DVE MAX8 instruction extracts
the block's top-8 v values (one pass).  The device returns all block
candidates [B, 8*nblocks]; the host gathers the 8 cores' candidates and
reduces to the global top-k.  A true top-k member can be missing only if
>=8 elements of its block rank above it, which forces >=8 of the
observed top-k to come from that single block - the host detects exactly
that condition and recomputes the affected rows with numpy, so the
selection is structurally sound for any k.
"""

import functools
import sys

sys.path.insert(0, "/opt/trn_rl_repo")

import numpy as np

P = 128
NCORES = 8
C2 = None  # set per-D at prep time: the constant pulled out of ||m||^2
PAD_V = -240.0  # fp8 value placed in the r rows of padding columns


def _ceil_to(x, m):
    return (x + m - 1) // m * m


def _chunk_widths(NPAD):
    """Column blocks per PSUM tile (each <=1024, >=8).  First block is
    small so the first matmuls start on minimal DMA; the last blocks are
    small so the post-matmul drain (copy+max8+out) is short."""
    assert NPAD >= 1024
    widths = [512]
    rem = NPAD - 512
    while rem > 1536:
        widths.append(1024)
        rem -= 1024
    if rem <= 519:
        widths.append(rem)
    elif rem <= 1024:
        widths += [512, rem - 512]
    elif rem < 1024 + 8:
        widths += [512, rem - 512]  # rem-512 in (512, 520): still <= 1024
    else:
        widths += [1024, rem - 1024]
    assert sum(widths) == NPAD and all(8 <= w <= 1024 for w in widths)
    return widths


@functools.lru_cache(maxsize=4)
def _build(B, D, NPAD):
    """Build (and finalize) the SPMD Bass module for one core's shard."""
    from contextlib import ExitStack

    import concourse.tile as tile
    from concourse import bacc, mybir

    f32 = mybir.dt.float32
    bf16 = mybir.dt.bfloat16
    f8 = mybir.dt.float8e4
    DR = mybir.MatmulPerfMode.DoubleRow

    KT = D // P
    KJ = KT // 2
    MT = B // P
    assert D % (2 * P) == 0 and B % P == 0 and NPAD >= 1024
    widths = _chunk_widths(NPAD)
    NCH = len(widths)
    CW = 8 * NCH  # candidates per row per core

    nc = bacc.Bacc(
        "TRN2", target_bir_lowering=False, debug=False, num_devices=NCORES
    )

    # host lays out both operands chunk-blocked and partition-contiguous
    # so DMAs move multi-KB whole-partition runs.  f is m-major so the
    # first matmul only needs the m=0 slice (~KT*P*P bytes).
    f_t = nc.declare_dram_parameter("f_t", [P, MT * KT * P], f8, isOutput=False)
    b_t = nc.declare_dram_parameter("b_t", [NCH, P, KT * 1024], f8, isOutput=False)
    out = nc.declare_dram_parameter("cand", [P, NCH * MT * 8], f32, isOutput=True)

    with tile.TileContext(nc) as tc, ExitStack() as ctx:
        cpool = ctx.enter_context(tc.tile_pool(name="const", bufs=1))
        # ring of 3 bank tiles: chunk ci's DMA is gated on chunk ci-3's
        # matmuls, so early chunks get the full DMA bandwidth instead of
        # fair-sharing with every later chunk's descriptors
        bpool = ctx.enter_context(tc.tile_pool(name="bank", bufs=3))
        ppool = ctx.enter_context(tc.tile_pool(name="psum", bufs=4, space="PSUM"))
        upool = ctx.enter_context(tc.tile_pool(name="u", bufs=6))

        b_t_view = b_t.rearrange("c p (kt n) -> c p kt n", n=1024)
        f_t_view = f_t.rearrange("p (mt kt b) -> p mt kt b", kt=KT, b=P)

        # PE warm-up during the initial DMA wait: garbage matmuls on a
        # zeroed tile get the HAM clock-gate toward 2.4GHz before real work
        warm = cpool.tile([P, 512], bf16, tag="warm")
        nc.vector.memset(warm[:], 0.0)
        wpsum = ppool.tile([P, 1024], f32, tag="pt")  # borrow a pt slot
        for _ in range(6):
            nc.tensor.matmul(
                wpsum[:, :512], lhsT=warm[:, :P], rhs=warm[:], start=True, stop=True
            )

        # features: one contiguous 6KB-per-partition DMA, issued first
        ftile = cpool.tile([P, MT, KT, P], f8, tag="ft")
        nc.scalar.dma_start(ftile[:], f_t_view[:])

        # bank chunk DMAs.  Chunk 0 (512 wide) is host-packed tightly as
        # [P, KT*512] so its DMA moves 3KB-per-partition runs and lands
        # right after f; the PE then streams with no early stall.
        b_t_view512 = b_t.rearrange("c p (kt n) -> c p kt n", n=512)
        bt0 = cpool.tile([P, KT, widths[0]], f8, tag="bt0")
        assert widths[0] == 512
        nc.sync.dma_start(bt0[:], b_t_view512[0, :, :KT, :])
        btiles = [bt0]
        for ci in range(1, NCH):
            W = widths[ci]
            btile = bpool.tile([P, KT, 1024], f8, tag="bt", name=f"bt{ci}")
            nc.sync.dma_start(btile[:, :, :W], b_t_view[ci, :, :, :W])
            btiles.append(btile)

        # candidates chunk-major: each chunk's slab DMAs out as soon as
        # its last MAX8 lands, overlapping output with compute
        cand = cpool.tile([P, NCH * MT * 8], f32, tag="cand")

        def chalves(W):
            out_, lo = [], 0
            while lo < W:
                out_.append((lo, min(512, W - lo)))
                lo += 512
            return out_

        for ci in range(NCH):
            W = widths[ci]
            for m in range(MT):
                pt = ppool.tile([P, 1024], f32, tag="pt", name=f"pt{ci}_{m}")
                for kj in range(KJ):
                    for hlo, hw in chalves(W):
                        nc.tensor.matmul(
                            pt[:, hlo : hlo + hw],
                            lhsT=ftile[:, m, 2 * kj : 2 * kj + 2, :],
                            rhs=btiles[ci][:, 2 * kj : 2 * kj + 2, hlo : hlo + hw],
                            start=(kj == 0),
                            stop=(kj == KJ - 1),
                            perf_mode=DR,
                        )
                u = upool.tile([P, 1024], f32, tag="u")
                nc.scalar.copy(u[:, :W], pt[:, :W])
                nc.vector.max(
                    cand[:, (ci * MT + m) * 8 : (ci * MT + m) * 8 + 8], u[:, :W]
                )
            # issue on the (idle) GpSimd queue: the Sync queue blocks on
            # ring-gated bank-chunk DMA issues until late in the run
            nc.gpsimd.dma_start(
                out[:, ci * MT * 8 : (ci + 1) * MT * 8],
                cand[:, ci * MT * 8 : (ci + 1) * MT * 8],
            )

    nc.finalize()
    return nc


def _host_prep(features, memory_bank):
    """Shard + lay out fp8 inputs for the 8 cores (chunk-blocked,
    partition-contiguous layouts so DMAs move multi-KB runs)."""
    import ml_dtypes

    f8 = ml_dtypes.float8_e4m3
    B, D = features.shape
    N = memory_bank.shape[0]
    DQ = D - 2  # data dims; last two carry the m-norm term
    KT = D // P
    MT = B // P
    NSH = -(-N // NCORES)
    NPAD = max(NSH, 1024)
    if NPAD % 1024 and NPAD % 1024 < 8:
        NPAD = _ceil_to(NPAD, 1024)
    widths = _chunk_widths(NPAD)
    NCH = len(widths)

    c2 = float(D)
    fT = np.empty((D, B), f8)
    fT[:DQ] = features[:, :DQ].T.astype(f8)
    fT[DQ:] = 1.0
    # [D, B] -> m-major [P, MT, KT, P]: element (p, m, kt, j) = fT[kt*P+p, m*P+j]
    f_t = np.ascontiguousarray(
        fT.reshape(KT, P, MT, P).transpose(1, 2, 0, 3).reshape(P, MT * KT * P)
    )
    x_sq = np.einsum("bd,bd->b", features, features, dtype=np.float32)

    msq = np.einsum("nd,nd->n", memory_bank, memory_bank, dtype=np.float32)
    r = 0.5 * (c2 - msq)
    r_hi = r.astype(f8)
    r_lo = (r - r_hi.astype(np.float32)).astype(f8)

    in_maps = []
    for i in range(NCORES):
        lo = i * NSH
        hi = min(lo + NSH, N)
        n_i = hi - lo
        bT = np.zeros((D, NPAD), f8)
        bT[:DQ, :n_i] = memory_bank[lo:hi, :DQ].T.astype(f8)
        bT[DQ, :n_i] = r_hi[lo:hi]
        bT[DQ + 1, :n_i] = r_lo[lo:hi]
        if n_i < NPAD:  # padding columns must rank below everything
            bT[DQ, n_i:] = PAD_V
            bT[DQ + 1, n_i:] = PAD_V
        # chunk-blocked: [NCH, P, KT*1024], unused tail cols left zero.
        # chunk 0 (512 wide) is packed tightly: [P, KT*512] runs.
        b_t = np.zeros((NCH, P, KT * 1024), f8)
        c0 = 0
        for ci, W in enumerate(widths):
            blk = bT[:, c0 : c0 + W].reshape(KT, P, W).transpose(1, 0, 2)
            if ci == 0:
                b_t[0, :, : KT * W] = blk.reshape(P, KT * W)
            else:
                b_t[ci].reshape(P, KT, 1024)[:, :, :W] = blk
            c0 += W
        in_maps.append({"f_t": f_t, "b_t": b_t})
    return in_maps, NPAD, x_sq, msq, c2


# test.py can flip these to get a profiled run
TRACE = False
LAST_RESULT = None
N_RECOMPUTED = 0


def _install_ntff_hook():
    """This container's `antenv` lacks `axon_hooks`; synthesize it so
    run_bass_kernel_spmd(trace=True) can profile via the axon .so."""
    import sys as _sys

    if "antenv.axon_hooks" in _sys.modules:
        return
    import contextlib, ctypes, types

    mod = types.ModuleType("antenv.axon_hooks")
    mod._hook = None
    mod.set_axon_ntff_profile_hook = lambda h: setattr(mod, "_hook", h)
    mod.get_axon_ntff_profile_hook = lambda: mod._hook

    so_path = "/opt/axon/libaxon_pjrt.so"
    try:
        lib = ctypes.CDLL(so_path)
        lib.axon_start_nrt_profile.argtypes = [
            ctypes.POINTER(ctypes.c_int64),
            ctypes.c_size_t,
        ]
        lib.axon_start_nrt_profile.restype = ctypes.c_int64
        lib.axon_stop_nrt_profile.argtypes = [ctypes.c_char_p]
        lib.axon_stop_nrt_profile.restype = ctypes.c_int64

        @contextlib.contextmanager
        def _hook(output_dir, device_ids):
            import jax

            jax.devices()
            if device_ids:
                ids = (ctypes.c_int64 * len(device_ids))(*device_ids)
                rc = lib.axon_start_nrt_profile(ids, len(device_ids))
            else:
                rc = lib.axon_start_nrt_profile(None, 0)
            if rc != 0:
                raise RuntimeError(f"axon_start_nrt_profile rc={rc}")
            try:
                yield
            finally:
                n = lib.axon_stop_nrt_profile(str(output_dir).encode())
                print(f"profile: {n} file(s) written to {output_dir}")

        mod._hook = _hook
    except (OSError, AttributeError):
        pass

    import antenv

    _sys.modules["antenv.axon_hooks"] = mod
    antenv.axon_hooks = mod


def _exact_row_scores(features, memory_bank, rows, kk):
    """Exact numpy top-k mean distance for a few suspect rows."""
    f = features[rows]  # [R, D]
    d2 = (
        np.einsum("rd,rd->r", f, f)[:, None]
        + np.einsum("nd,nd->n", memory_bank, memory_bank)[None, :]
        - 2.0 * (f @ memory_bank.T)
    )
    d2k = np.sort(d2, axis=1)[:, :kk]
    return np.sqrt(np.maximum(d2k, 0.0)).mean(axis=1)


def kernel(features, memory_bank, k):
    global LAST_RESULT, N_RECOMPUTED
    from concourse.bass_utils import run_bass_kernel_spmd

    features = np.asarray(features, dtype=np.float32)
    memory_bank = np.asarray(memory_bank, dtype=np.float32)
    B, D = features.shape
    N = memory_bank.shape[0]
    kk = min(int(k), N)
    if kk <= 0:
        # mean over an empty candidate set (matches jnp.mean of empty)
        return np.full(B, np.nan, np.float32)

    in_maps, NPAD, x_sq, msq, c2 = _host_prep(features, memory_bank)
    nc = _build(B, D, NPAD)

    if TRACE:
        _install_ntff_hook()
    res = run_bass_kernel_spmd(nc, in_maps, list(range(NCORES)), trace=TRACE)
    LAST_RESULT = res

    # gather per-(core, block) top-8 candidates; larger v = closer.
    # device layout is [P, NCH*MT*8]: row p, col (ci*MT+m)*8+j -> feature
    # row m*P+p, block ci, candidate j
    MT = B // P
    cols = []
    for i in range(NCORES):
        arr = np.asarray(res.results[i]["cand"], dtype=np.float32)
        NCH = arr.shape[1] // (MT * 8)
        cols.append(
            arr.reshape(P, NCH, MT, 8).transpose(2, 0, 1, 3).reshape(B, NCH * 8)
        )
    v = np.concatenate(cols, axis=1)  # [B, NCORES * 8 * nblocks]
    return _finalize(v, features, memory_bank, kk, x_sq, c2)


def _finalize(v, features, memory_bank, kk, x_sq, c2):
    """Reduce the per-(core, block) top-8 candidates to the final scores."""
    global N_RECOMPUTED
    kk_c = min(kk, v.shape[1])
    order = np.argsort(-v, axis=1)[:, :kk_c]  # observed top-k candidates
    vk = np.take_along_axis(v, order, axis=1)
    d = np.sqrt(np.maximum(x_sq[:, None] + c2 - 2.0 * vk, 0.0))
    scores = d.mean(axis=1).astype(np.float32)

    # A true top-k member can only be missing if >=8 elements of its
    # 1024-column block outrank it; then >=8 of the observed top-k come
    # from that block (index group of 8).  Recompute such rows exactly.
    N_RECOMPUTED = 0
    if kk >= 9:
        if kk > v.shape[1]:  # more than the candidate pool: all rows exact
            suspects = np.arange(v.shape[0])
        else:
            grp = np.sort(order // 8, axis=1)
            same8 = (grp[:, 7:] == grp[:, : grp.shape[1] - 7]).any(axis=1)
            suspects = np.nonzero(same8)[0]
        if suspects.size:
            N_RECOMPUTED = suspects.size
            scores[suspects] = _exact_row_scores(
                features, memory_bank, suspects, kk
            ).astype(np.float32)

    return scores
